# revision 36
# baseline (speedup 1.0000x reference)
"""Trainium2 Bass kernel for nn_BasicMTL_49014166781960 (coref-style NMS detection).

Pipeline:
  Phase A (8 cores, candidates sharded 512/core): windowed softmax attention
    pooling + span embedding assembly (transposed/feature-major) + unary
    mention MLP -> mention_scores, span_embT.
  Host: greedy non-crossing span selection (sequential scan, exact replica of
    the reference), top-mention gather.
  Phase C (8 cores, w_fast columns sharded 384/core + a 20-col strip whose
    row-sharded contribution each core computes for its own 128 mention rows):
    fast-antecedent bilinear GEMM partial products -> host sum + unary/mask
    adds + per-row top-k (desc, ties -> lowest index, matching lax.top_k).
All matmuls fp32 (score ordering requires fp32-level accuracy).
"""
import sys
sys.path.insert(0, "/opt/trn_rl_repo")
import numpy as np
import concourse.bass as bass
import concourse.bacc as bacc
import concourse.mybir as mybir
import concourse.tile as tile
from concourse.bass_utils import run_bass_kernel_spmd

F32 = mybir.dt.float32
I32 = mybir.dt.int32
AF = mybir.ActivationFunctionType
OP = mybir.AluOpType

T, C, H, W_MAX, META, UH = 4096, 4096, 1024, 30, 20, 1024
SPAN_DIM = 3 * H + META          # 3092
DPAD = 3200                      # padded span dim (25 * 128)
NCORE = 8
CS = C // NCORE                  # 512 candidates per core
TLOC = 768                       # local token window (6 * 128)
NTT = TLOC // 128                # 6 token tiles
M = 1024                         # num_top_mentions
KANT = 50                        # num_top_antecedents
SHARD = 384                      # w_fast col shard (8*384=3072; +20-col strip)
NEG = np.float32(-1e30)

_TRACE = [False]
_DBG = {}
_HW_NS = None


def _build_phase_a(tsets):
    nc = bacc.Bacc("TRN2")
    P = 128
    hid_in = nc.dram_tensor("hid", [TLOC, H], F32, kind="ExternalInput")
    wattn_in = nc.dram_tensor("wattn", [P, H], F32, kind="ExternalInput")
    srep_in = nc.dram_tensor("srep", [P, CS], F32, kind="ExternalInput")
    erep_in = nc.dram_tensor("erep", [P, CS], F32, kind="ExternalInput")
    wdrep_in = nc.dram_tensor("wdrep", [P, CS], F32, kind="ExternalInput")
    tok_in = nc.dram_tensor("tokc", [P, NTT], F32, kind="ExternalInput")
    sidx_in = nc.dram_tensor("sidx", [P, CS // P], I32, kind="ExternalInput")
    eidx_in = nc.dram_tensor("eidx", [P, CS // P], I32, kind="ExternalInput")
    wu1_in = nc.dram_tensor("wu1", [SPAN_DIM, UH], F32, kind="ExternalInput")
    wu2_in = nc.dram_tensor("wu2", [P, UH // P], F32, kind="ExternalInput")
    bu1_in = nc.dram_tensor("bu1", [P, UH // P], F32, kind="ExternalInput")
    wwid_in = nc.dram_tensor("wwid", [P, META], F32, kind="ExternalInput")
    ones_in = nc.dram_tensor("ones", [P, P], F32, kind="ExternalInput")
    ident_in = nc.dram_tensor("ident", [P, P], F32, kind="ExternalInput")

    embT_out = nc.dram_tensor("embT", [DPAD, CS], F32, kind="ExternalOutput")
    sc_out = nc.dram_tensor("scores", [1, CS], F32, kind="ExternalOutput")

    NSP = CS // P  # 4 span tiles

    with tile.TileContext(nc) as tc:
        with tc.tile_pool(name="cst", bufs=1) as cst, \
             tc.tile_pool(name="hidp", bufs=1) as hidp, \
             tc.tile_pool(name="wtp", bufs=1) as wtp, \
             tc.tile_pool(name="gp", bufs=2) as gp, \
             tc.tile_pool(name="big", bufs=1) as bigp, \
             tc.tile_pool(name="wu", bufs=12) as wup, \
             tc.tile_pool(name="psb", bufs=7, space="PSUM") as psb, \
             tc.tile_pool(name="pss", bufs=1, space="PSUM") as pss:

            wattn = cst.tile([P, H], F32)
            nc.sync.dma_start(wattn[:], wattn_in[:])
            srep = cst.tile([P, CS], F32)
            nc.sync.dma_start(srep[:], srep_in[:])
            erep = cst.tile([P, CS], F32)
            nc.sync.dma_start(erep[:], erep_in[:])
            wdrep = cst.tile([P, CS], F32)
            nc.sync.dma_start(wdrep[:], wdrep_in[:])
            tokc = cst.tile([P, NTT], F32)
            nc.sync.dma_start(tokc[:], tok_in[:])
            sidx = cst.tile([P, NSP], I32)
            nc.sync.dma_start(sidx[:], sidx_in[:])
            eidx = cst.tile([P, NSP], I32)
            nc.sync.dma_start(eidx[:], eidx_in[:])
            wu2 = cst.tile([P, UH // P], F32)
            nc.sync.dma_start(wu2[:], wu2_in[:])
            bu1 = cst.tile([P, UH // P], F32)
            nc.sync.dma_start(bu1[:], bu1_in[:])
            wwid = cst.tile([P, META], F32)
            nc.sync.dma_start(wwid[:], wwid_in[:])
            ones = cst.tile([P, P], F32)
            nc.sync.dma_start(ones[:], ones_in[:])
            ident = cst.tile([P, P], F32)
            nc.sync.dma_start(ident[:], ident_in[:])

            hid = []
            for t in range(NTT):
                ht = hidp.tile([P, H], F32, tag=f"hid{t}")
                nc.sync.dma_start(ht[:], hid_in[t * P:(t + 1) * P, :])
                hid.append(ht)

            # gather hidden[starts], hidden[ends] span-major then PE-transpose
            sT = [bigp.tile([P, CS], F32, tag=f"sT{m}", name=f"sT{m}") for m in range(8)]
            eT = [bigp.tile([P, CS], F32, tag=f"eT{m}", name=f"eT{m}") for m in range(8)]
            for i in range(NSP):
                gs = gp.tile([P, H], F32, tag="gs")
                nc.gpsimd.indirect_dma_start(
                    out=gs[:], out_offset=None, in_=hid_in[:],
                    in_offset=bass.IndirectOffsetOnAxis(ap=sidx[:, i:i + 1], axis=0))
                geg = gp.tile([P, H], F32, tag="geg")
                nc.gpsimd.indirect_dma_start(
                    out=geg[:], out_offset=None, in_=hid_in[:],
                    in_offset=bass.IndirectOffsetOnAxis(ap=eidx[:, i:i + 1], axis=0))
                for m in range(8):
                    pt = pss.tile([P, P], F32, tag="pt")
                    nc.tensor.transpose(pt[:], gs[:, m * P:(m + 1) * P], ident[:])
                    nc.vector.tensor_copy(sT[m][:, i * P:(i + 1) * P], pt[:])
                    pt2 = pss.tile([P, P], F32, tag="pt")
                    nc.tensor.transpose(pt2[:], geg[:, m * P:(m + 1) * P], ident[:])
                    nc.vector.tensor_copy(eT[m][:, i * P:(i + 1) * P], pt2[:])
            for m in range(8):
                nc.sync.dma_start(embT_out[m * P:(m + 1) * P, :], sT[m][:])
                nc.sync.dma_start(embT_out[H + m * P:H + (m + 1) * P, :], eT[m][:])

            # width one-hot + w_width.T block [META, CS]
            oh = bigp.tile([P, CS], F32, tag="m2")
            nc.vector.tensor_scalar(out=oh[:], in0=wdrep[:],
                                    scalar1=tokc[:, 0:1], scalar2=0.0,
                                    op0=OP.subtract, op1=OP.is_equal)
            pw = psb.tile([P, CS], F32, tag="ps")
            nc.tensor.matmul(pw[:META, :], wwid[:], oh[:], start=True, stop=True)
            wTt = bigp.tile([P, CS], F32, tag="rec2")
            nc.vector.tensor_copy(wTt[:META, :], pw[:META, :])
            nc.sync.dma_start(embT_out[2 * H:2 * H + META, :], wTt[:META, :])

            # token weights p_t = exp(logit_t), per token tile
            pcol = cst.tile([P, NTT], F32)
            for t in sorted({t for s in tsets for t in s}):
                scr = bigp.tile([P, H], F32, tag="scr")
                nc.vector.tensor_tensor(out=scr[:], in0=hid[t][:], in1=wattn[:],
                                        op=OP.mult)
                lg = bigp.tile([P, 1], F32, tag=f"lg{t}")
                nc.vector.reduce_sum(out=lg[:], in_=scr[:], axis=mybir.AxisListType.X)
                nc.scalar.activation(pcol[:, t:t + 1], lg[:], AF.Exp)

            # banded attention weights Wt[t] = (tok>=s)&(tok<=e)*p_t
            used_t = sorted({t for s in tsets for t in s})
            wt = {}
            for t in used_t:
                m1 = wtp.tile([P, CS], F32, tag=f"m1_{t}")
                nc.vector.tensor_scalar(out=m1[:], in0=srep[:],
                                        scalar1=tokc[:, t:t + 1], scalar2=0.0,
                                        op0=OP.subtract, op1=OP.is_le)
                m2 = bigp.tile([P, CS], F32, tag="m2")
                nc.vector.tensor_scalar(out=m2[:], in0=erep[:],
                                        scalar1=tokc[:, t:t + 1], scalar2=0.0,
                                        op0=OP.subtract, op1=OP.is_ge)
                nc.vector.tensor_tensor(out=m1[:], in0=m1[:], in1=m2[:], op=OP.mult)
                nc.vector.tensor_scalar_mul(m1[:], in0=m1[:], scalar1=pcol[:, t:t + 1])
                wt[t] = m1

            # denominators (replicated column sums) + reciprocal
            pd = psb.tile([P, CS], F32, tag="ps")
            for i in range(NSP):
                ts_i = tsets[i]
                for j, t in enumerate(ts_i):
                    nc.tensor.matmul(pd[:, i * P:(i + 1) * P], ones[:],
                                     wt[t][:, i * P:(i + 1) * P],
                                     start=(j == 0), stop=(j == len(ts_i) - 1))
            rec = bigp.tile([P, CS], F32, tag="rec")
            nc.vector.reciprocal(rec[:], pd[:])

            # attendedT [H, CS] feature-major, normalized (banded over token tiles)
            attT = []
            for m in range(8):
                pa = psb.tile([P, CS], F32, tag="ps")
                for i in range(NSP):
                    ts_i = tsets[i]
                    for j, t in enumerate(ts_i):
                        nc.tensor.matmul(pa[:, i * P:(i + 1) * P],
                                         hid[t][:, m * P:(m + 1) * P],
                                         wt[t][:, i * P:(i + 1) * P],
                                         start=(j == 0), stop=(j == len(ts_i) - 1))
                at = bigp.tile([P, CS], F32, tag=f"attT{m}")
                nc.vector.tensor_tensor(out=at[:], in0=pa[:], in1=rec[:], op=OP.mult)
                nc.sync.dma_start(embT_out[2 * H + META + m * P:
                                           2 * H + META + (m + 1) * P, :], at[:])
                attT.append(at)

            # unary MLP: u1 = relu(span_emb @ w_u1 + b_u1) computed transposed
            pieces = ([(sT[k], k * P, P) for k in range(8)]
                      + [(eT[k], H + k * P, P) for k in range(8)]
                      + [(wTt, 2 * H, META)]
                      + [(attT[k], 2 * H + META + k * P, P) for k in range(8)])
            relu = []
            for m in range(8):
                pu = psb.tile([P, CS], F32, tag="ps")
                for j, (rtile, row0, kk) in enumerate(pieces):
                    wtile = wup.tile([P, P], F32, tag="wu1")
                    nc.sync.dma_start(wtile[:kk, :],
                                      wu1_in[row0:row0 + kk, m * P:(m + 1) * P])
                    nc.tensor.matmul(pu[:], wtile[:kk, :], rtile[:kk, :],
                                     start=(j == 0), stop=(j == len(pieces) - 1))
                rl = bigp.tile([P, CS], F32, tag=f"relu{m}")
                nc.scalar.activation(rl[:], pu[:], AF.Relu, bias=bu1[:, m:m + 1])
                relu.append(rl)

            # mention scores = relu @ w_u2  (+b_u2 added on host; it's 0 anyway)
            psc = psb.tile([P, CS], F32, tag="ps")
            for m in range(8):
                nc.tensor.matmul(psc[:1, :], wu2[:, m:m + 1], relu[m][:],
                                 start=(m == 0), stop=(m == 7))
            scs = bigp.tile([P, CS], F32, tag="scs")
            nc.vector.tensor_copy(scs[:1, :], psc[:1, :])
            nc.sync.dma_start(sc_out[:, :], scs[:1, :])

    nc.compile()
    return nc


def _build_phase_c(nuniq):
    nc = bacc.Bacc("TRN2")
    P = 128
    NK = DPAD // P            # 25 k tiles
    topT_in = nc.dram_tensor("topT", [DPAD, M], F32, kind="ExternalInput")
    wfs_in = nc.dram_tensor("wfs", [DPAD, SHARD], F32, kind="ExternalInput")
    wfstrip_in = nc.dram_tensor("wfstrip", [DPAD, META], F32, kind="ExternalInput")
    topkb_in = nc.dram_tensor("topkb", [DPAD, P], F32, kind="ExternalInput")
    tsh_in = nc.dram_tensor("tshard", [SHARD, M], F32, kind="ExternalInput")
    z_out = nc.dram_tensor("zpart", [M, M], F32, kind="ExternalOutput")
    zx_out = nc.dram_tensor("zx", [P, M], F32, kind="ExternalOutput")
    N2 = nuniq - 512          # second mention-chunk width (512 < nuniq <= 1024)

    with tile.TileContext(nc) as tc:
        with tc.tile_pool(name="topp", bufs=1) as topp, \
             tc.tile_pool(name="wfp", bufs=12) as wfp, \
             tc.tile_pool(name="yp", bufs=1) as yp, \
             tc.tile_pool(name="shp", bufs=1) as shp, \
             tc.tile_pool(name="zp", bufs=3) as zp, \
             tc.tile_pool(name="ps", bufs=7, space="PSUM") as ps, \
             tc.tile_pool(name="psx", bufs=1, space="PSUM") as psx:

            topT = []
            for k in range(NK):
                tt = topp.tile([P, M], F32, tag=f"topT{k}", name=f"topT{k}")
                nc.gpsimd.dma_start(tt[:], topT_in[k * P:(k + 1) * P, :])
                topT.append(tt)

            # gemm1: Y^T [SHARD, M]
            ysb = []
            for mi in range(3):
                ym = yp.tile([P, M], F32, tag=f"y{mi}", name=f"y{mi}")
                pyt0 = ps.tile([P, 512], F32, tag="ps")
                pyt1 = ps.tile([P, 512], F32, tag="ps")
                for k in range(NK):
                    wtile = wfp.tile([P, P], F32, tag="wf", name="wtile")
                    nc.sync.dma_start(
                        wtile[:],
                        wfs_in[k * P:(k + 1) * P, mi * P:(mi + 1) * P])
                    nc.tensor.matmul(pyt0[:], wtile[:], topT[k][:, 0:512],
                                     start=(k == 0), stop=(k == NK - 1))
                    nc.tensor.matmul(pyt1[:, :N2], wtile[:],
                                     topT[k][:, 512:512 + N2],
                                     start=(k == 0), stop=(k == NK - 1))
                nc.vector.tensor_copy(ym[:, 0:512], pyt0[:])
                nc.vector.tensor_copy(ym[:, 512:512 + N2], pyt1[:, :N2])
                ysb.append(ym)

            # strip (last 20 real w_fast cols): Y_extra^T [20, 128-row-block]
            pyx = psx.tile([P, P], F32, tag="px", name="pyx")
            for k in range(NK):
                wst = wfp.tile([P, META], F32, tag="wst", name="wst")
                nc.sync.dma_start(wst[:], wfstrip_in[k * P:(k + 1) * P, :])
                tkb = wfp.tile([P, P], F32, tag="tkb", name="tkb")
                nc.sync.dma_start(tkb[:], topkb_in[k * P:(k + 1) * P, :])
                nc.tensor.matmul(pyx[:META, :], wst[:], tkb[:],
                                 start=(k == 0), stop=(k == NK - 1))
            yx = zp.tile([P, P], F32, tag="yx", name="yx")
            nc.vector.tensor_copy(yx[:META, :], pyx[:META, :])
            # Z_extra rows (this core's 128 mention rows) over all antecedents
            for n, nw in ((0, 512), (1, N2)):
                pzx = psx.tile([P, 512], F32, tag="px", name="pzx")
                nc.tensor.matmul(pzx[:, :nw], yx[:META, :],
                                 topT[24][:META, n * 512:n * 512 + nw],
                                 start=True, stop=True)
                zxs = zp.tile([P, 512], F32, tag="zxs", name="zxs")
                nc.vector.tensor_copy(zxs[:, :nw], pzx[:, :nw])
                nc.sync.dma_start(zx_out[:, n * 512:n * 512 + nw], zxs[:, :nw])

            # shard rows of topT for gemm2 rhs (separate aligned input)
            tsh = []
            for mi in range(3):
                th = shp.tile([P, M], F32, tag=f"tsh{mi}", name=f"tsh{mi}")
                nc.gpsimd.dma_start(th[:], tsh_in[mi * P:(mi + 1) * P, :])
                tsh.append(th)

            # gemm2: Z[r, j] += sum_d Y^T[d, r] * topT_shard[d, j]
            for r in range(8):
                for n, nw in ((0, 512), (1, N2)):
                    pz = ps.tile([P, 512], F32, tag="ps")
                    for mi in range(3):
                        nc.tensor.matmul(pz[:, :nw],
                                         ysb[mi][:, r * P:(r + 1) * P],
                                         tsh[mi][:, n * 512:n * 512 + nw],
                                         start=(mi == 0), stop=(mi == 2))
                    zt = zp.tile([P, 512], F32, tag="zt", name="zt")
                    nc.vector.tensor_copy(zt[:, :nw], pz[:, :nw])
                    nc.sync.dma_start(
                        z_out[r * P:(r + 1) * P, n * 512:n * 512 + nw],
                        zt[:, :nw])

    nc.compile()
    return nc


def _greedy_scan(starts, ends, scores, num_top, n_tokens):
    order = np.argsort(-scores, kind="stable")
    latest_end = np.full((n_tokens,), -1, np.int64)
    earliest_start = np.full((n_tokens,), n_tokens, np.int64)
    count = 0
    flags = np.zeros(len(starts), bool)
    for ind in order:
        s_i, e_i = int(starts[ind]), int(ends[ind])
        cross1 = np.any(latest_end[s_i + 1:e_i + 1] > e_i)
        cross2 = np.any(earliest_start[s_i:e_i] < s_i)
        if (not (cross1 or cross2)) and (count < num_top):
            latest_end[s_i] = max(latest_end[s_i], e_i)
            earliest_start[e_i] = min(earliest_start[e_i], s_i)
            count += 1
            flags[ind] = True
    sel = np.nonzero(flags)[0]
    out = np.full((num_top,), len(starts), np.int64)
    out[:min(num_top, len(sel))] = sel[:num_top]
    return np.minimum(out, len(starts) - 1).astype(np.int32), min(num_top, len(sel))


def kernel(hidden_states, span_starts, span_widths, num_top_mentions,
           num_top_antecedents, w_width, w_attn, b_attn, w_u1, b_u1, w_u2,
           b_u2, w_fast, b_fast):
    hs = np.ascontiguousarray(np.asarray(hidden_states, np.float32))
    starts = np.asarray(span_starts).astype(np.int64)
    widths = np.asarray(span_widths).astype(np.int64)
    ends = starts + widths
    num_top = int(num_top_mentions)
    kant = int(num_top_antecedents)
    w_width = np.asarray(w_width, np.float32)
    w_attn = np.asarray(w_attn, np.float32)
    b_attn_f = np.float32(b_attn)
    w_u1 = np.ascontiguousarray(np.asarray(w_u1, np.float32))
    b_u1 = np.asarray(b_u1, np.float32)
    w_u2 = np.asarray(w_u2, np.float32)
    b_u2f = np.float32(np.asarray(b_u2).reshape(-1)[0])
    w_fast = np.asarray(w_fast, np.float32)
    b_fast = np.asarray(b_fast, np.float32)
    P = 128

    # ---- phase A host prep ----
    lo = np.zeros(NCORE, np.int64)
    in_maps_a = []
    eye = np.eye(P, dtype=np.float32)
    onesm = np.ones((P, P), np.float32)
    tokc = (np.arange(P, dtype=np.float32)[:, None]
            + 128.0 * np.arange(NTT, dtype=np.float32)[None, :])
    wattn_rep = np.repeat(w_attn[None, :], P, 0)
    wu2c = np.ascontiguousarray(w_u2[:, 0].reshape(UH // P, P).T)
    bu1c = np.ascontiguousarray(b_u1.reshape(UH // P, P).T)
    wwid_pad = np.zeros((P, META), np.float32)
    wwid_pad[:W_MAX, :] = w_width
    for c in range(NCORE):
        s_sh = starts[c * CS:(c + 1) * CS]
        e_sh = ends[c * CS:(c + 1) * CS]
        w_sh = widths[c * CS:(c + 1) * CS]
        lo_c = int(s_sh.min())
        lo_c = min(lo_c, T - TLOC)
        assert int(e_sh.max()) < lo_c + TLOC, "local token window overflow"
        lo[c] = lo_c
        hid_loc = hs[lo_c:lo_c + TLOC]
        sl = (s_sh - lo_c).astype(np.float32)
        el = (e_sh - lo_c).astype(np.float32)
        in_maps_a.append(dict(
            hid=np.ascontiguousarray(hid_loc),
            wattn=wattn_rep,
            srep=np.repeat(sl[None, :], P, 0),
            erep=np.repeat(el[None, :], P, 0),
            wdrep=np.repeat(w_sh.astype(np.float32)[None, :], P, 0),
            tokc=tokc,
            sidx=np.ascontiguousarray((s_sh - lo_c).astype(np.int32).reshape(CS // P, P).T),
            eidx=np.ascontiguousarray((e_sh - lo_c).astype(np.int32).reshape(CS // P, P).T),
            wu1=w_u1, wu2=wu2c, bu1=bu1c, wwid=wwid_pad,
            ones=onesm, ident=eye,
        ))

    tsets = []
    for i in range(C // NCORE // P):
        lo_t, hi_t = NTT, -1
        for c in range(NCORE):
            s_sh = starts[c * CS:(c + 1) * CS] - lo[c]
            e_sh = ends[c * CS:(c + 1) * CS] - lo[c]
            lo_t = min(lo_t, int(s_sh[i * 128:(i + 1) * 128].min()) // 128)
            hi_t = max(hi_t, int(e_sh[i * 128:(i + 1) * 128].max()) // 128)
        tsets.append(list(range(lo_t, hi_t + 1)))
    nc_a = _build_phase_a(tuple(tuple(s) for s in tsets))
    _DBG["nc_a"] = nc_a
    res_a = run_bass_kernel_spmd(nc_a, in_maps_a, core_ids=list(range(NCORE)),
                                 trace=_TRACE[0])
    scores = np.concatenate([r["scores"][0] for r in res_a.results]) + b_u2f
    embT_all = np.concatenate([r["embT"] for r in res_a.results], axis=1)

    # ---- host: greedy non-crossing selection ----
    top_idx, nacc = _greedy_scan(starts, ends, scores, num_top, T)
    nuniq = max(513, min(M, nacc + 1))
    _DBG["scores"] = scores
    _DBG["embT"] = embT_all
    _DBG["top_idx"] = top_idx

    top_embT = np.ascontiguousarray(embT_all[:, top_idx])   # [DPAD, M]
    top_m = scores[top_idx]

    # ---- phase C host prep ----
    wf_pad = np.zeros((DPAD, SPAN_DIM), np.float32)
    wf_pad[:SPAN_DIM, :] = w_fast
    wfstrip = np.ascontiguousarray(wf_pad[:, 8 * SHARD:SPAN_DIM])  # [DPAD, 20]
    in_maps_c = []
    for c in range(NCORE):
        in_maps_c.append(dict(
            topT=top_embT,
            wfs=np.ascontiguousarray(wf_pad[:, c * SHARD:(c + 1) * SHARD]),
            wfstrip=wfstrip,
            topkb=np.ascontiguousarray(top_embT[:, c * 128:(c + 1) * 128]),
            tshard=np.ascontiguousarray(top_embT[c * SHARD:(c + 1) * SHARD, :]),
        ))
    nc_c = _build_phase_c(nuniq)
    _DBG["nc_c"] = nc_c
    res_c = run_bass_kernel_spmd(nc_c, in_maps_c, core_ids=list(range(NCORE)),
                                 trace=_TRACE[0])
    gemm = res_c.results[0]["zpart"].astype(np.float32)
    for c in range(1, NCORE):
        gemm = gemm + res_c.results[c]["zpart"]
    for c in range(NCORE):
        gemm[c * 128:(c + 1) * 128, :] += res_c.results[c]["zx"]
    if nuniq < M:
        # rows/cols beyond the first padded mention are bitwise duplicates
        gemm[:, nuniq:] = gemm[:, nuniq - 1:nuniq]
        gemm[nuniq:, :] = gemm[nuniq - 1:nuniq, :]
    # b_fast contribution: fast[r, j] += sum_d b_fast[d] * top_emb[j, d]
    bterm = (top_embT[:SPAN_DIM, :].T @ b_fast).astype(np.float32)  # [M]

    rng = np.arange(M)
    ant_mask_full = (rng[:, None] - rng[None, :]) >= 1
    fast = (top_m[:, None] + top_m[None, :]
            + np.where(ant_mask_full, np.float32(0.0), NEG)
            + gemm + bterm[None, :]).astype(np.float32)

    # top-k (desc, ties -> lowest index) per row
    _DBG["fast"] = fast
    idx = np.argsort(-fast, axis=1, kind="stable")[:, :kant]
    top_ant = idx.astype(np.int32)
    top_ant_mask = np.take_along_axis(ant_mask_full, idx, axis=1)
    top_ant_scores = np.take_along_axis(fast, idx, axis=1)
    return (top_idx.astype(np.int32), top_ant, top_ant_mask, top_ant_scores)
